# revision 1
# baseline (speedup 1.0000x reference)
"""Trainium2 Bass kernel for nn_Attention_32195074851105.

Pipeline per core (data-parallel over N=8192 rows, 1024 rows/core):
  emb gather (indirect DMA from bf16 table) -> DMA-transpose ->
  conv as shifted-filter-bank matmuls (feature-major output) -> FC1 -> FC2 ->
  gating projections -> softmax over 2 = sigmoid(diff) -> scale ld tensors.

All engine SBUF access patterns start at partition 0 (HW constraint:
engine APs may only start at partitions {0,32,64,96} with width caps).
The conv's sliding window misalignment is absorbed into per-group
shifted filter-bank variants, built on device with matmuls against a
sliding identity and streamed from DRAM.

Self-contained: hardcodes shapes, runs on 8 NeuronCores via
run_bass_kernel_spmd, gathers full outputs.
"""

import sys

if "/opt/trn_rl_repo" not in sys.path:
    sys.path.insert(0, "/opt/trn_rl_repo")

import numpy as np

import concourse.bass as bass
import concourse.bacc as bacc
import concourse.mybir as mybir
import concourse.tile as tile
from concourse.bass import IndirectOffsetOnAxis
from concourse.bass_utils import run_bass_kernel_spmd

AF = mybir.ActivationFunctionType

F32 = mybir.dt.float32
BF16 = mybir.dt.bfloat16
I32 = mybir.dt.int32

N_CORES = 8
N = 8192
R = N // N_CORES  # rows per core
RB = 512          # rows per block
V, E, EP = 645, 1140, 1152   # emb vocab, emb dim, padded emb dim (9*128)
CH, KW, SW, J = 32, 25, 9, 124  # conv channels, kernel w, stride, out positions
G = 4             # conv output positions per 128-feature chunk (32ch * 4pos)
NCH = J // G      # 31 feature chunks of 128
WIN = KW + SW * (G - 1)  # 52-wide input window per chunk
FEAT = CH * J     # 3968
H1, H2, D = 1000, 100, 512
ALPHA = 0.01      # leaky relu slope
USE_HW_LRELU = True  # sim doesn't implement Lrelu; flip off for CoreSim runs


def conv_pieces(g):
    """For group g: list of (emb_tile_index, identity_slice_start) pieces.

    Window taps [36g, 36g+52). Piece 1 reads full emb tile t0 with the bank
    shifted down by a = 36g % 128 (variant rows [a, a+52) hold the taps).
    Piece 2 (when the window spills into tile t0+1) holds taps [128-a, 52)
    at variant rows [0, a+52-128).
    """
    u0 = SW * G * g
    t0, a = divmod(u0, 128)
    out = [(t0, 128 - a)]
    if a + WIN > 128:
        out.append((t0 + 1, 256 - a))
    return out


# distinct identity-slice starts across all groups/pieces, in a fixed order
VOFFS = sorted({off for g in range(NCH) for _, off in conv_pieces(g)})
VIDX = {off: i for i, off in enumerate(VOFFS)}
NVAR = len(VOFFS)


def emit_lrelu(nc, sb, out_ap, psum_ap, bias_ap, tag):
    """out = leaky_relu(psum + bias). 1 ACT op on HW; 3-op fallback for sim."""
    if USE_HW_LRELU:
        nc.scalar.activation(out=out_ap, in_=psum_ap, func=AF.Lrelu, bias=bias_ap, alpha=ALPHA)
    else:
        shape = [128, psum_ap.shape[-1]]
        u = sb.tile(shape, BF16, tag="lr_u", bufs=1, name=f"lru_{tag}")
        u = u[: psum_ap.shape[0]]
        nc.scalar.activation(out=u[:], in_=psum_ap, func=AF.Identity, bias=bias_ap)
        v = sb.tile(shape, BF16, tag="lr_v", bufs=1, name=f"lrv_{tag}")
        v = v[: psum_ap.shape[0]]
        nc.vector.tensor_scalar_mul(out=v[:], in0=u[:], scalar1=ALPHA)
        nc.vector.tensor_tensor(out=out_ap, in0=u[:], in1=v[:], op=mybir.AluOpType.max)


def build_graph(rows=R):
    nblk = rows // RB
    rt_per_blk = RB // 128
    nrt = rows // 128

    nc = bacc.Bacc(
        "TRN2",
        target_bir_lowering=False,
        debug=False,
        num_devices=N_CORES,
    )
    p = {}
    p["ld_gcn"] = nc.declare_dram_parameter("ld_gcn", [rows, D], F32, isOutput=False)
    p["ld_encoder"] = nc.declare_dram_parameter("ld_encoder", [rows, D], F32, isOutput=False)
    p["x"] = nc.declare_dram_parameter("x", [rows], I32, isOutput=False)
    p["y"] = nc.declare_dram_parameter("y", [rows], I32, isOutput=False)
    p["H_emb"] = nc.declare_dram_parameter("H_emb", [V, E], F32, isOutput=False)
    p["conv_w"] = nc.declare_dram_parameter("conv_w", [CH, 1, 2, KW], F32, isOutput=False)
    p["conv_b"] = nc.declare_dram_parameter("conv_b", [CH], F32, isOutput=False)
    p["W1"] = nc.declare_dram_parameter("W1", [H1, FEAT], F32, isOutput=False)
    p["b1"] = nc.declare_dram_parameter("b1", [H1], F32, isOutput=False)
    p["W2"] = nc.declare_dram_parameter("W2", [H2, H1], F32, isOutput=False)
    p["b2"] = nc.declare_dram_parameter("b2", [H2], F32, isOutput=False)
    p["Wg"] = nc.declare_dram_parameter("Wg", [H2, D], F32, isOutput=False)
    p["bg"] = nc.declare_dram_parameter("bg", [H2], F32, isOutput=False)
    p["We"] = nc.declare_dram_parameter("We", [H2, D], F32, isOutput=False)
    p["be"] = nc.declare_dram_parameter("be", [H2], F32, isOutput=False)
    out = nc.declare_dram_parameter("out", [2 * rows, D], F32, isOutput=True)

    with tile.TileContext(nc) as tc:
        build_body(nc, tc, p, out[:], rows, nblk, rt_per_blk, nrt)
    nc.compile()
    return nc


def build_body(nc, tc, p, out, rows, nblk, rt_per_blk, nrt):
    with (
        tc.tile_pool(name="sb", bufs=1) as sb,
        tc.tile_pool(name="ps", bufs=1, space="PSUM") as psp,
        tc.tile_pool(name="dr", bufs=1, space="DRAM") as drp,
    ):
        # ---------------- one-time prep ----------------
        ones = sb.tile([128, 1], BF16, tag="ones", bufs=1)
        nc.vector.memset(ones[:], 1.0)
        negones = sb.tile([128, 1], BF16, tag="negones", bufs=1)
        nc.vector.memset(negones[:], -1.0)

        # row indices on partitions: xi[p, t] = x[t*128+p]
        xi = sb.tile([128, nrt], I32, tag="xi", bufs=1)
        nc.sync.dma_start(out=xi[:], in_=p["x"][:].rearrange("(t q) -> q t", q=128))
        yi = sb.tile([128, nrt], I32, tag="yi", bufs=1)
        nc.sync.dma_start(out=yi[:], in_=p["y"][:].rearrange("(t q) -> q t", q=128))
        yp = sb.tile([128, nrt], I32, tag="yp", bufs=1)
        nc.vector.tensor_scalar_add(out=yp[:], in0=yi[:], scalar1=240)

        # master conv filter bank built in SBUF via shift-matmuls:
        # bank[u, 128*h + o*4+jl] = conv_w[o,0,h,u-9*jl].
        # Per-jl base tiles hold taps at rows [0,25) in their own column
        # subset; accumulating matmuls against a sliding f32 identity shift
        # each by 9*jl. Engines (unlike DMA) have no semaphore-wait limits.
        Iw = sb.tile([128, 320], BF16, tag="Iw", bufs=1)
        nc.gpsimd.memset(Iw[:], 0.0)
        nc.gpsimd.affine_select(
            out=Iw[:], in_=Iw[:], compare_op=mybir.AluOpType.not_equal,
            fill=1.0, base=128, pattern=[[-1, 320]], channel_multiplier=1,
        )
        # conv_w loaded contiguously, transposed on PE, scattered with DVE
        ident = sb.tile([CH, CH], F32, tag="ident", bufs=1)
        from concourse.masks import make_identity
        make_identity(nc, ident[:])
        cw_sb = sb.tile([CH, 2 * KW], F32, tag="cw_sb", bufs=1)
        nc.scalar.dma_start(out=cw_sb[:], in_=p["conv_w"][:, 0, :, :])
        cw_pad = sb.tile([CH, 64], F32, tag="cw_pad", bufs=1)
        nc.vector.memset(cw_pad[:], 0.0)
        nc.vector.tensor_copy(out=cw_pad[:, 0:KW], in_=cw_sb[:, 0:KW])
        nc.vector.tensor_copy(out=cw_pad[:, 32 : 32 + KW], in_=cw_sb[:, KW : 2 * KW])
        tp_ps = psp.tile([64, CH], F32, tag="smallps", bufs=2, name="cwtp")
        nc.tensor.transpose(out=tp_ps[:], in_=cw_pad[:], identity=ident[:])
        cwT = sb.tile([64, CH], BF16, tag="cwT", bufs=1)
        nc.scalar.activation(out=cwT[:], in_=tp_ps[:], func=AF.Copy)
        base_jls = []
        for jl in range(G):
            bjl = sb.tile([128, 256], BF16, tag=f"bjl{jl}", bufs=1, name=f"bjl{jl}")
            nc.vector.memset(bjl[:], 0.0)
            for h in (0, 1):
                nc.vector.tensor_copy(
                    out=bjl[:KW, 128 * h : 128 * (h + 1)].rearrange("w (o j) -> w o j", j=G)[:, :, jl],
                    in_=cwT[32 * h : 32 * h + KW, :],
                )
            base_jls.append(bjl)
        psb = psp.tile([128, 256], F32, tag="convps", bufs=2, name="psbank")
        for jl in range(G):
            nc.tensor.matmul(
                psb[:], lhsT=Iw[:, 128 - SW * jl : 256 - SW * jl], rhs=base_jls[jl][:],
                start=(jl == 0), stop=(jl == 3),
            )
        bank = sb.tile([128, 256], BF16, tag="bank", bufs=1)
        nc.scalar.activation(out=bank[:], in_=psb[:], func=AF.Copy)

        # shifted bank variants -> DRAM (streamed back per block).
        # variant[off][v, m] = bank[v - (128 - off), m]
        vdram = drp.tile([NVAR, 128, 256], BF16, tag="vdram", bufs=1)
        for off in VOFFS:
            vp = psp.tile([128, 256], F32, tag="vps", bufs=2, name=f"vps{off}")
            nc.tensor.matmul(vp[:], lhsT=Iw[:, off : off + 128], rhs=bank[:], start=True, stop=True)
            vs = sb.tile([128, 256], BF16, tag="vstage", bufs=3, name=f"vs{off}")
            nc.scalar.activation(out=vs[:], in_=vp[:], func=AF.Copy)
            nc.sync.dma_start(out=vdram[VIDX[off]], in_=vs[:])

        # conv bias per partition via DRAM bounce: cb[p] = conv_b[p//4]
        # cb[p] = conv_b[p//4] via matmul against an expand matrix
        cbt0 = sb.tile([CH, 1], F32, tag="cbt0", bufs=1)
        nc.sync.dma_start(out=cbt0[:], in_=p["conv_b"][:].rearrange("o -> o ()"))
        expand = sb.tile([CH, 128], F32, tag="expand", bufs=1)
        nc.vector.memset(expand[:], 0.0)
        for jj in range(G):
            nc.vector.tensor_copy(
                out=expand.rearrange("q (c j) -> q c j", j=G)[:, :, jj], in_=ident[:]
            )
        cb_ps = psp.tile([128, 1], F32, tag="smallps", bufs=2, name="cbps")
        nc.tensor.matmul(cb_ps[:], lhsT=expand[:], rhs=cbt0[:], start=True, stop=True)
        cb = sb.tile([128, 1], F32, tag="cb", bufs=1)
        nc.scalar.activation(out=cb[:], in_=cb_ps[:], func=AF.Copy)

        # fc biases on partitions
        b1v = sb.tile([128, 8], F32, tag="b1v", bufs=1)
        nc.sync.dma_start(out=b1v[:, :7], in_=p["b1"][0:896].rearrange("(c q) -> q c", q=128))
        nc.sync.dma_start(out=b1v[:104, 7:8], in_=p["b1"][896:1000].rearrange("(c q) -> q c", q=104))
        b2v = sb.tile([128, 1], F32, tag="b2v", bufs=1)
        nc.sync.dma_start(out=b2v[:100, :], in_=p["b2"][:].rearrange("q -> q ()"))
        bgv = sb.tile([128, 1], F32, tag="bgv", bufs=1)
        nc.sync.dma_start(out=bgv[:100, :], in_=p["bg"][:].rearrange("q -> q ()"))
        bev = sb.tile([128, 1], F32, tag="bev", bufs=1)
        nc.sync.dma_start(out=bev[:100, :], in_=p["be"][:].rearrange("q -> q ()"))

        # W1T: k-major bf16, feature order permuted to (g, o, jl) chunks.
        # W1T[p, g, mt, c] = W1[mt*128+c, 124*(p//4) + 4*g + (p%4)]
        W1T = sb.tile([128, NCH, 8, 128], BF16, tag="W1T", bufs=1)

        def emit_w1prep():
          for mt in range(8):
              m0 = mt * 128
              mr = min(128, H1 - m0)
              wf = sb.tile([128, FEAT], F32, tag="stage_f", bufs=1, name=f"w1f{mt}")
              nc.scalar.dma_start(out=wf[:mr], in_=p["W1"][m0 : m0 + mr, :])
              wb = sb.tile([128, FEAT], BF16, tag="stage_b", bufs=1, name=f"w1b{mt}")
              if mr < 128:
                  nc.vector.memset(wb[96:], 0.0)
              nc.vector.tensor_copy(
                  out=wb[:mr].rearrange("m (g o j) -> m g o j", g=NCH, o=CH, j=G),
                  in_=wf[:mr].rearrange("m (o g j) -> m g o j", o=CH, g=NCH, j=G),
              )
              nc.scalar.dma_start(out=W1T[:, :, mt, :], in_=wb[:], transpose=True)

        # W2T[p, kt, c] = W2[c, kt*128+p]
        w2f = sb.tile([H2, H1], F32, tag="stage_f", bufs=1, name="w2f")
        nc.scalar.dma_start(out=w2f[:], in_=p["W2"][:])
        w2b = sb.tile([128, 1024], BF16, tag="stage_b", bufs=1)
        nc.vector.memset(w2b[:], 0.0)
        nc.vector.tensor_copy(out=w2b[:H2, :H1], in_=w2f[:])
        W2T = sb.tile([128, 8, 128], BF16, tag="W2T", bufs=1)
        nc.scalar.dma_start(out=W2T[:], in_=w2b[:], transpose=True)

        # WgT/WeT[p, kt, c] = W[c, kt*128+p]
        WgT = sb.tile([128, 4, 128], BF16, tag="WgT", bufs=1)
        WeT = sb.tile([128, 4, 128], BF16, tag="WeT", bufs=1)
        for wname, wdst in (("Wg", WgT), ("We", WeT)):
            wgf = sb.tile([128, D], F32, tag="stage_g", bufs=1, name=f"{wname}f")
            nc.vector.memset(wgf[:], 0.0)
            nc.scalar.dma_start(out=wgf[:H2, :], in_=p[wname][:])
            wgb = sb.tile([128, D], BF16, tag="stage_gb", bufs=1, name=f"{wname}b")
            nc.vector.tensor_copy(out=wgb[:], in_=wgf[:])
            nc.scalar.dma_start(out=wdst[:], in_=wgb[:], transpose=True)

        # ---------------- steady state (software-pipelined blocks) ----------------
        def emit_head(b):
            t = {}
            t["embxT"] = sb.tile([128, 9, rt_per_blk, 128], BF16, tag="embxT", bufs=1, name=f"embxT{b}")
            t["embyT"] = sb.tile([128, 9, rt_per_blk, 128], BF16, tag="embyT", bufs=1, name=f"embyT{b}")
            t["ldgb"] = sb.tile([128, rt_per_blk * D], BF16, tag="ldgb", bufs=1, name=f"ldgb{b}")
            t["ldeb"] = sb.tile([128, rt_per_blk * D], BF16, tag="ldeb", bufs=1, name=f"ldeb{b}")
            ldgT = sb.tile([128, 4, rt_per_blk, 128], BF16, tag="ldgT", bufs=1, name=f"ldgT{b}")
            ldeT = sb.tile([128, 4, rt_per_blk, 128], BF16, tag="ldeT", bufs=1, name=f"ldeT{b}")
            for rt in range(rt_per_blk):
                bt = b * rt_per_blk + rt
                for nm, ldb_, ldT_ in (("ld_gcn", t["ldgb"], ldgT), ("ld_encoder", t["ldeb"], ldeT)):
                    lf = sb.tile([128, D], F32, tag="ldf", bufs=2, name=f"lf_{nm}{bt}")
                    nc.sync.dma_start(out=lf[:], in_=p[nm][bt * 128 : (bt + 1) * 128, :])
                    nc.vector.tensor_copy(out=ldb_[:, rt * D : (rt + 1) * D], in_=lf[:])
                    nc.sync.dma_start(
                        out=ldT_[:, :, rt, :], in_=ldb_[:, rt * D : (rt + 1) * D],
                        transpose=True,
                    )
                for pref, idx_, ekey in (("gx", xi, "embxT"), ("gy", yp, "embyT")):
                    gf = sb.tile([128, E], F32, tag="gxyf", bufs=2, name=f"{pref}f{bt}")
                    nc.gpsimd.indirect_dma_start(
                        out=gf[:], out_offset=None, in_=p["H_emb"][:],
                        in_offset=IndirectOffsetOnAxis(ap=idx_[:, bt : bt + 1], axis=0),
                    )
                    gb = sb.tile([128, EP], BF16, tag="gxyb", bufs=2, name=f"{pref}b{bt}")
                    nc.vector.tensor_copy(out=gb[:, :E], in_=gf[:])
                    nc.vector.memset(gb[:, E:], 0.0)
                    nc.sync.dma_start(out=t[ekey][:, :, rt, :], in_=gb[:], transpose=True)

            # gating projections (transposed domain): gT = tanh(WgT.T @ ldT + bg)
            for nm, ldT_, bv in (("gT", ldgT, bgv), ("eT", ldeT, bev)):
                WT_ = WgT if nm == "gT" else WeT
                psg = psp.tile([128, RB], F32, tag="smallps", bufs=2, name=f"ps_{nm}{b}")
                for kt in range(4):
                    nc.tensor.matmul(
                        psg[:H2], lhsT=WT_[:, kt, :H2], rhs=ldT_[:, kt, :, :],
                        start=(kt == 0), stop=(kt == 3),
                    )
                gt = sb.tile([H2, RB], BF16, tag=nm, bufs=2, name=f"{nm}{b}")
                nc.scalar.activation(out=gt[:], in_=psg[:H2], func=AF.Tanh, bias=bv[:H2, :])
                t[nm] = gt

            # conv -> cT chunks (feature-major, 128 features x RB rows)
            cT = sb.tile([128, NCH, RB], BF16, tag="cT", bufs=1, name=f"cT{b}")
            for g in range(NCH):
                ps = psp.tile([128, RB], F32, tag="convps", bufs=2, name=f"cps{b}_{g}")
                pieces = conv_pieces(g)
                vts = []
                for tt, off in pieces:
                    vt = sb.tile([128, 256], BF16, tag="vt", bufs=3, name=f"vt{b}_{g}_{off}")
                    nc.sync.dma_start(out=vt[:], in_=vdram[VIDX[off]])
                    vts.append((tt, vt))
                nmm = 2 * len(vts)
                i = 0
                for half, ekey in ((0, "embxT"), (1, "embyT")):
                    for tt, vt in vts:
                        nc.tensor.matmul(
                            ps[:], lhsT=vt[:, 128 * half : 128 * half + 128],
                            rhs=t[ekey][:, tt, :, :],
                            start=(i == 0), stop=(i == nmm - 1),
                        )
                        i += 1
                emit_lrelu(nc, sb, cT[:, g, :], ps[:], cb[:, :], "c")
            t["cT"] = cT
            return t

        def emit_tail(b, t):
            cT = t["cT"]
            hfc1T = sb.tile([128, 8, RB], BF16, tag="hfc1T", bufs=1, name=f"hfc1T{b}")
            for mc in range(8):
                mw = min(128, H1 - mc * 128)
                ps = psp.tile([128, RB], F32, tag="fc1ps", bufs=2, name=f"fps{b}_{mc}")
                for kt in range(NCH):
                    nc.tensor.matmul(
                        ps[:mw], lhsT=W1T[:, kt, mc, :mw], rhs=cT[:, kt, :],
                        start=(kt == 0), stop=(kt == NCH - 1),
                    )
                emit_lrelu(nc, sb, hfc1T[:mw, mc, :], ps[:mw], b1v[:mw, mc : mc + 1], "f1")

            ps2 = psp.tile([128, RB], F32, tag="smallps", bufs=2, name=f"ps2_{b}")
            for kt in range(8):
                kw = min(128, H1 - kt * 128)
                nc.tensor.matmul(
                    ps2[:H2], lhsT=W2T[:kw, kt, :H2], rhs=hfc1T[:kw, kt, :],
                    start=(kt == 0), stop=(kt == 7),
                )
            hfcT = sb.tile([H2, RB], BF16, tag="hfcT", bufs=2, name=f"hfcT{b}")
            emit_lrelu(nc, sb, hfcT[:], ps2[:H2], b2v[:H2, :], "f2")

            pg = sb.tile([H2, RB], BF16, tag="pg", bufs=2, name=f"pg{b}")
            nc.vector.tensor_tensor(out=pg[:], in0=t["gT"][:], in1=hfcT[:], op=mybir.AluOpType.mult)
            pe = sb.tile([H2, RB], BF16, tag="pe", bufs=2, name=f"pe{b}")
            nc.vector.tensor_tensor(out=pe[:], in0=t["eT"][:], in1=hfcT[:], op=mybir.AluOpType.mult)
            psd = psp.tile([1, RB], F32, tag="smallps", bufs=2, name=f"psd{b}")
            nc.tensor.matmul(psd[:], lhsT=ones[:H2, :], rhs=pg[:], start=True, stop=False)
            nc.tensor.matmul(psd[:], lhsT=negones[:H2, :], rhs=pe[:], start=False, stop=True)

            attp = sb.tile([64, RB], BF16, tag="attp", bufs=2, name=f"attp{b}")
            nc.vector.memset(attp[:], 0.0)
            nc.scalar.activation(out=attp[0:1, :], in_=psd[:], func=AF.Sigmoid)
            nc.scalar.activation(out=attp[32:33, :], in_=psd[:], func=AF.Sigmoid, scale=-1.0)
            attT = sb.tile([128, rt_per_blk, 64], BF16, tag="attT", bufs=2, name=f"attT{b}")
            nc.sync.dma_start(out=attT[:], in_=attp[:], transpose=True)
            attTf = sb.tile([128, rt_per_blk, 2], F32, tag="attTf", bufs=2, name=f"attTf{b}")
            nc.vector.tensor_copy(out=attTf[:, :, 0:1], in_=attT[:, :, 0:1])
            nc.vector.tensor_copy(out=attTf[:, :, 1:2], in_=attT[:, :, 32:33])

            for rt in range(rt_per_blk):
                bt = b * rt_per_blk + rt
                og = sb.tile([128, D], F32, tag="oo", bufs=2, name=f"og{bt}")
                nc.vector.tensor_scalar_mul(
                    out=og[:], in0=t["ldgb"][:, rt * D : (rt + 1) * D],
                    scalar1=attTf[:, rt, 0:1],
                )
                nc.sync.dma_start(out=out[bt * 128 : (bt + 1) * 128, :], in_=og[:])
                oe = sb.tile([128, D], F32, tag="oo", bufs=2, name=f"oe{bt}")
                nc.vector.tensor_scalar_mul(
                    out=oe[:], in0=t["ldeb"][:, rt * D : (rt + 1) * D],
                    scalar1=attTf[:, rt, 1:2],
                )
                nc.sync.dma_start(out=out[rows + bt * 128 : rows + (bt + 1) * 128, :], in_=oe[:])

        for b in range(nblk):
            cur = emit_head(b)
            if b == 0:
                emit_w1prep()
            emit_tail(b, cur)


_CACHED = {}


def _get_graph(rows=R):
    if rows not in _CACHED:
        _CACHED[rows] = build_graph(rows)
    return _CACHED[rows]


def kernel(**inputs):
    nc = _get_graph(R)
    in_maps = []
    for c in range(N_CORES):
        sl = slice(c * R, (c + 1) * R)
        m = {
            "ld_gcn": np.ascontiguousarray(inputs["ld_gcn"][sl]).astype(np.float32, copy=False),
            "ld_encoder": np.ascontiguousarray(inputs["ld_encoder"][sl]).astype(np.float32, copy=False),
            "x": np.ascontiguousarray(inputs["x"][sl]).astype(np.int32),
            "y": np.ascontiguousarray(inputs["y"][sl]).astype(np.int32),
        }
        for k in ("H_emb", "conv_w", "conv_b", "W1", "b1", "W2", "b2", "Wg", "bg", "We", "be"):
            m[k] = np.ascontiguousarray(np.asarray(inputs[k], dtype=np.float32))
        in_maps.append(m)
    res = run_bass_kernel_spmd(nc, in_maps, core_ids=list(range(N_CORES)))
    outs = [r["out"] for r in res.results]
    out1 = np.concatenate([o[:R] for o in outs], axis=0)
    out2 = np.concatenate([o[R:] for o in outs], axis=0)
    return out1, out2


if __name__ == "__main__":
    nc = build_graph()
    print("graph built OK")



# revision 11
# speedup vs baseline: 1.8241x; 1.8241x over previous
"""Trainium2 Bass kernel for nn_Attention_32195074851105.

Pipeline per core (data-parallel over N=8192 rows, 1024 rows/core):
  emb gather (indirect DMA from host-prepped bf16 table) -> DMA-transpose ->
  conv as shifted-filter-bank matmuls (feature-major output) -> FC1 -> FC2 ->
  gating projections -> softmax over 2 = sigmoid(diff) -> scale ld tensors.

All constant tensors (permuted W1/W2/Wg/We, conv filter-bank variants,
bias layouts, bf16-padded embedding table) and the ld transposes are
precomputed on the host in numpy, so the device graph has no prologue
beyond a handful of straight DMA loads. Engine DMA queues (sync,
scalar, gpsimd) are assigned so that block b+1's gathers/transposes
overlap block b's matmuls; output DMAs are deferred past the next
block's transposes to keep the sync queue free.

Self-contained: hardcodes shapes, runs on 8 NeuronCores via
run_bass_kernel_spmd, gathers full outputs.
"""

import sys

if "/opt/trn_rl_repo" not in sys.path:
    sys.path.insert(0, "/opt/trn_rl_repo")

import numpy as np
import ml_dtypes

import concourse.bass as bass
import concourse.bacc as bacc
import concourse.mybir as mybir
import concourse.tile as tile
from concourse.bass import IndirectOffsetOnAxis
from concourse.bass_utils import run_bass_kernel_spmd

AF = mybir.ActivationFunctionType

F32 = mybir.dt.float32
BF16 = mybir.dt.bfloat16
I32 = mybir.dt.int32
BF = ml_dtypes.bfloat16

N_CORES = 8
N = 8192
R = N // N_CORES     # rows per core
RB = 512             # rows per block
NBLK = R // RB       # 2
RT = RB // 128       # row-tiles per block
NRT = R // 128       # row-tiles per core
V, E, EP = 645, 1140, 1152     # emb vocab, emb dim, padded emb dim (9*128)
CH, KW, SW, J = 32, 25, 9, 124 # conv channels, kernel w, stride, out positions
G = 4                # conv output positions per 128-feature group
NCH = J // G         # 31 feature groups of 128
WIN = KW + SW * (G - 1)  # 52-wide input window per group
H1, H2, D = 1000, 100, 512
MW = 125             # H1 chunk width (8 chunks of 125, no padding)
ALPHA = 0.01         # leaky relu slope


def conv_pieces(g):
    """For group g: list of (emb_tile_index, variant_shift s) pieces.

    Window taps [36g, 36g+52). s = 36g - 128*t places the variant's
    taps at partition rows [s + 9*jl + k]. A second piece (next tile,
    s-128) is needed when the window crosses a 128 boundary.
    """
    t0, a = divmod(SW * G * g, 128)
    out = [(t0, a)]
    if a + WIN > 128:
        out.append((t0 + 1, a - 128))
    return out


SVALS = sorted({s for g in range(NCH) for _, s in conv_pieces(g)})
SIDX = {s: i for i, s in enumerate(SVALS)}
NVAR = len(SVALS)


# ---------------------------------------------------------------- host prep

def _shared_prep(inputs):
    f32 = np.float32
    H = np.asarray(inputs["H_emb"], f32)
    Hp = np.zeros((V, EP), BF)
    Hp[:, :E] = H.astype(BF)

    w = np.asarray(inputs["conv_w"], f32)  # [32,1,2,25]
    vb = np.zeros((128, NVAR, 256), f32)
    ovec = np.arange(CH) * G
    for si, s in enumerate(SVALS):
        for h in (0, 1):
            for jl in range(G):
                for k in range(KW):
                    v = s + SW * jl + k
                    if 0 <= v < 128:
                        vb[v, si, 128 * h + ovec + jl] = w[:, 0, h, k]
    vbank = vb.reshape(128, NVAR * 256).astype(BF)

    W1 = np.asarray(inputs["W1"], f32)  # [1000, 3968]
    # W1T[p=(o,j), g, mt, c] = W1[mt*125+c, o*124 + g*4 + j]
    W1T = (
        W1.reshape(8, MW, CH, NCH, G)
        .transpose(2, 4, 3, 0, 1)
        .reshape(128, NCH * 8 * MW)
        .astype(BF)
    )
    W2 = np.asarray(inputs["W2"], f32)  # [100, 1000]
    W2T = W2.T.reshape(8, MW, H2).transpose(1, 0, 2).reshape(MW, 8 * H2).astype(BF)

    def gateT(Wm):
        Wp = np.zeros((128, D), f32)
        Wp[:H2] = np.asarray(Wm, f32)
        return Wp.T.reshape(4, 128, 128).transpose(1, 0, 2).reshape(128, 512).astype(BF)

    biases = np.zeros((128, 12), f32)
    b1 = np.asarray(inputs["b1"], f32)
    for mt in range(8):
        biases[:MW, mt] = b1[mt * MW : (mt + 1) * MW]
    biases[:, 8] = np.asarray(inputs["conv_b"], f32)[np.arange(128) // G]
    biases[:H2, 9] = np.asarray(inputs["b2"], f32)
    biases[:H2, 10] = np.asarray(inputs["bg"], f32)
    biases[:H2, 11] = np.asarray(inputs["be"], f32)

    return {
        "H_emb": Hp,
        "vbank": vbank,
        "W1T": W1T,
        "W2T": W2T,
        "WgT": gateT(inputs["Wg"]),
        "WeT": gateT(inputs["We"]),
        "biases": biases,
    }


def _ldT(ld):
    # [128, 4*R]: ldT[p, kt, r] = ld[r, kt*128+p]
    return np.ascontiguousarray(
        ld.T.reshape(4, 128, R).transpose(1, 0, 2).reshape(128, 4 * R)
    )


def make_in_maps(inputs):
    shared = _shared_prep(inputs)
    x = np.asarray(inputs["x"]).astype(np.int32)
    y = np.asarray(inputs["y"]).astype(np.int32) + 240
    ldg = np.asarray(inputs["ld_gcn"], np.float32).astype(BF)
    lde = np.asarray(inputs["ld_encoder"], np.float32).astype(BF)
    maps = []
    for c in range(N_CORES):
        sl = slice(c * R, (c + 1) * R)
        m = dict(shared)
        m["x_idx"] = np.ascontiguousarray(x[sl])
        m["y_idx"] = np.ascontiguousarray(y[sl])
        m["ldTg"] = _ldT(ldg[sl])
        m["ldTe"] = _ldT(lde[sl])
        m["ldbg"] = np.ascontiguousarray(ldg[sl])
        m["ldbe"] = np.ascontiguousarray(lde[sl])
        maps.append(m)
    return maps


# ---------------------------------------------------------------- graph

def build_graph():
    nc = bacc.Bacc(
        "TRN2",
        target_bir_lowering=False,
        debug=False,
        num_devices=N_CORES,
    )
    p = {}

    def par(name, shape, dt):
        p[name] = nc.declare_dram_parameter(name, shape, dt, isOutput=False)

    par("x_idx", [R], I32)
    par("y_idx", [R], I32)
    par("H_emb", [V, EP], BF16)
    par("vbank", [128, NVAR * 256], BF16)
    par("W1T", [128, NCH * 8 * MW], BF16)
    par("W2T", [MW, 8 * H2], BF16)
    par("WgT", [128, 512], BF16)
    par("WeT", [128, 512], BF16)
    par("biases", [128, 12], F32)
    par("ldTg", [128, 4 * R], BF16)
    par("ldTe", [128, 4 * R], BF16)
    par("ldbg", [R, D], BF16)
    par("ldbe", [R, D], BF16)
    out = nc.declare_dram_parameter("out", [2 * R, D], BF16, isOutput=True)

    with tile.TileContext(nc) as tc:
        build_body(nc, tc, p, out[:])
    nc.compile()
    return nc


def build_body(nc, tc, p, out):
    with (
        tc.tile_pool(name="sb", bufs=1) as sb,
        tc.tile_pool(name="ps", bufs=1, space="PSUM") as psp,
    ):
        # ------------- prologue loads (small; W1T halves come later) -------
        xi = sb.tile([128, NRT], I32, tag="xi", bufs=1)
        nc.sync.dma_start(out=xi[:], in_=p["x_idx"][:].rearrange("(t q) -> q t", q=128))
        yi = sb.tile([128, NRT], I32, tag="yi", bufs=1)
        nc.sync.dma_start(out=yi[:], in_=p["y_idx"][:].rearrange("(t q) -> q t", q=128))

        vb = sb.tile([128, NVAR, 256], BF16, tag="vb", bufs=1)
        nc.scalar.dma_start(
            out=vb[:], in_=p["vbank"][:].rearrange("p (n c) -> p n c", c=256)
        )
        WgT = sb.tile([128, 4, 128], BF16, tag="WgT", bufs=1)
        nc.scalar.dma_start(
            out=WgT[:], in_=p["WgT"][:].rearrange("p (k c) -> p k c", c=128)
        )
        WeT = sb.tile([128, 4, 128], BF16, tag="WeT", bufs=1)
        nc.scalar.dma_start(
            out=WeT[:], in_=p["WeT"][:].rearrange("p (k c) -> p k c", c=128)
        )
        bia = sb.tile([128, 12], F32, tag="bia", bufs=1)
        nc.scalar.dma_start(out=bia[:], in_=p["biases"][:])
        W2T = sb.tile([MW, 8, H2], BF16, tag="W2T", bufs=1)
        nc.scalar.dma_start(
            out=W2T[:], in_=p["W2T"][:].rearrange("p (k c) -> p k c", c=H2)
        )

        ones = sb.tile([128, 1], BF16, tag="ones", bufs=1)
        nc.vector.memset(ones[:], 1.0)
        negones = sb.tile([128, 1], BF16, tag="negones", bufs=1)
        nc.vector.memset(negones[:], -1.0)

        W1T = sb.tile([128, NCH, 8, MW], BF16, tag="W1T", bufs=1)
        HG = 15  # groups in W1T half 1

        def emit_w1_h1():  # gpsimd queue, after block-0 gathers
            nc.gpsimd.dma_start(
                out=W1T[:, :HG],
                in_=p["W1T"][:, : HG * 8 * MW].rearrange(
                    "p (g m c) -> p g m c", m=8, c=MW
                ),
            )

        def emit_w1_h2():  # sync queue, after block-0 transposes
            nc.sync.dma_start(
                out=W1T[:, HG:],
                in_=p["W1T"][:, HG * 8 * MW :].rearrange(
                    "p (g m c) -> p g m c", m=8, c=MW
                ),
            )

        def emit_ldT(b):  # gpsimd queue, after block-b gathers
            ldTg = sb.tile([128, 4, RB], BF16, tag="ldTg", bufs=1, name=f"ldTg{b}")
            nc.gpsimd.dma_start(
                out=ldTg[:],
                in_=p["ldTg"][:].rearrange("p (k r) -> p k r", r=R)[
                    :, :, b * RB : (b + 1) * RB
                ],
            )
            ldTe = sb.tile([128, 4, RB], BF16, tag="ldTe", bufs=1, name=f"ldTe{b}")
            nc.gpsimd.dma_start(
                out=ldTe[:],
                in_=p["ldTe"][:].rearrange("p (k r) -> p k r", r=R)[
                    :, :, b * RB : (b + 1) * RB
                ],
            )
            return ldTg, ldTe

        # ------------- steady state ---------------------------------------
        def emit_head(b):
            t = {}
            t["embxT"] = sb.tile([128, 9, RT, 128], BF16, tag="embxT", bufs=2,
                                 name=f"embxT{b}")
            t["embyT"] = sb.tile([128, 9, RT, 128], BF16, tag="embyT", bufs=2,
                                 name=f"embyT{b}")
            for rt in range(RT):
                bt = b * RT + rt
                for pref, idx_, ekey in (("gx", xi, "embxT"), ("gy", yi, "embyT")):
                    gf = sb.tile([128, EP], BF16, tag=pref, bufs=2,
                                 name=f"{pref}{bt}")
                    nc.gpsimd.indirect_dma_start(
                        out=gf[:], out_offset=None, in_=p["H_emb"][:],
                        in_offset=IndirectOffsetOnAxis(ap=idx_[:, bt : bt + 1], axis=0),
                    )
                    nc.sync.dma_start(out=t[ekey][:, :, rt, :], in_=gf[:],
                                      transpose=True)
            ldTg, ldTe = emit_ldT(b)

            # conv -> cT groups (feature-major, 128 features x RB rows)
            cT = sb.tile([128, NCH, RB], BF16, tag="cT", bufs=1, name=f"cT{b}")
            for g in range(NCH):
                ps = psp.tile([128, RB], F32, tag="convps", bufs=2, name=f"cps{b}_{g}")
                pieces = conv_pieces(g)
                nmm = 2 * len(pieces)
                i = 0
                for half, ekey in ((0, "embxT"), (1, "embyT")):
                    for tt, s in pieces:
                        nc.tensor.matmul(
                            ps[:],
                            lhsT=vb[:, SIDX[s], 128 * half : 128 * half + 128],
                            rhs=t[ekey][:, tt, :, :],
                            start=(i == 0), stop=(i == nmm - 1),
                        )
                        i += 1
                nc.scalar.activation(out=cT[:, g, :], in_=ps[:], func=AF.Lrelu,
                                     bias=bia[:, 8:9], alpha=ALPHA)
            t["cT"] = cT

            # gating projections: gT = tanh(WgT.T @ ldT + bg)
            for nm, WT_, ldT_, bcol in (("gT", WgT, ldTg, 10), ("eT", WeT, ldTe, 11)):
                psg = psp.tile([128, RB], F32, tag="smallps", bufs=2,
                               name=f"ps_{nm}{b}")
                for kt in range(4):
                    nc.tensor.matmul(
                        psg[:H2], lhsT=WT_[:, kt, :H2], rhs=ldT_[:, kt, :],
                        start=(kt == 0), stop=(kt == 3),
                    )
                gt = sb.tile([H2, RB], BF16, tag=nm, bufs=2, name=f"{nm}{b}")
                nc.scalar.activation(out=gt[:], in_=psg[:H2], func=AF.Tanh,
                                     bias=bia[:H2, bcol : bcol + 1])
                t[nm] = gt
            return t

        def emit_tail(b, t):
            # ld row-major chunks for the output scaling (scalar queue)
            lds = []
            for rt in range(RT):
                bt = b * RT + rt
                lg = sb.tile([128, D], BF16, tag="lgb", bufs=4, name=f"lg{bt}")
                nc.scalar.dma_start(out=lg[:], in_=p["ldbg"][bt * 128 : (bt + 1) * 128, :])
                le = sb.tile([128, D], BF16, tag="leb", bufs=4, name=f"le{bt}")
                nc.scalar.dma_start(out=le[:], in_=p["ldbe"][bt * 128 : (bt + 1) * 128, :])
                lds.append((lg, le))

            cT = t["cT"]
            hfc1T = sb.tile([128, 8, RB], BF16, tag="hfc1T", bufs=1, name=f"hfc1T{b}")
            for mc in range(8):
                ps = psp.tile([128, RB], F32, tag="fc1ps", bufs=2, name=f"fps{b}_{mc}")
                for kt in range(NCH):
                    nc.tensor.matmul(
                        ps[:MW], lhsT=W1T[:, kt, mc, :], rhs=cT[:, kt, :],
                        start=(kt == 0), stop=(kt == NCH - 1),
                    )
                nc.scalar.activation(out=hfc1T[:MW, mc, :], in_=ps[:MW],
                                     func=AF.Lrelu, bias=bia[:MW, mc : mc + 1],
                                     alpha=ALPHA)

            ps2 = psp.tile([128, RB], F32, tag="smallps", bufs=2, name=f"ps2_{b}")
            for kt in range(8):
                nc.tensor.matmul(
                    ps2[:H2], lhsT=W2T[:, kt, :], rhs=hfc1T[:MW, kt, :],
                    start=(kt == 0), stop=(kt == 7),
                )
            hfcT = sb.tile([H2, RB], BF16, tag="hfcT", bufs=2, name=f"hfcT{b}")
            nc.scalar.activation(out=hfcT[:], in_=ps2[:H2], func=AF.Lrelu,
                                 bias=bia[:H2, 9:10], alpha=ALPHA)

            pg = sb.tile([H2, RB], BF16, tag="pg", bufs=2, name=f"pg{b}")
            nc.vector.tensor_tensor(out=pg[:], in0=t["gT"][:], in1=hfcT[:],
                                    op=mybir.AluOpType.mult)
            pe = sb.tile([H2, RB], BF16, tag="pe", bufs=2, name=f"pe{b}")
            nc.vector.tensor_tensor(out=pe[:], in0=t["eT"][:], in1=hfcT[:],
                                    op=mybir.AluOpType.mult)
            psd = psp.tile([1, RB], F32, tag="smallps", bufs=2, name=f"psd{b}")
            nc.tensor.matmul(psd[:], lhsT=ones[:H2, :], rhs=pg[:], start=True,
                             stop=False)
            nc.tensor.matmul(psd[:], lhsT=negones[:H2, :], rhs=pe[:], start=False,
                             stop=True)

            attp = sb.tile([64, RB], BF16, tag="attp", bufs=2, name=f"attp{b}")
            nc.scalar.activation(out=attp[0:1, :], in_=psd[:], func=AF.Sigmoid)
            nc.scalar.activation(out=attp[32:33, :], in_=psd[:], func=AF.Sigmoid,
                                 scale=-1.0)
            attT = sb.tile([128, RT, 64], BF16, tag="attT", bufs=2, name=f"attT{b}")
            nc.scalar.dma_start(out=attT[:], in_=attp[:], transpose=True)
            attTf = sb.tile([128, RT, 2], F32, tag="attTf", bufs=2, name=f"attTf{b}")
            nc.vector.tensor_copy(out=attTf[:, :, 0:1], in_=attT[:, :, 0:1])
            nc.vector.tensor_copy(out=attTf[:, :, 1:2], in_=attT[:, :, 32:33])

            # output scaling in place; DMAs deferred (emitted after next
            # head's transposes so they queue behind them on sync)
            outs = []
            for rt in range(RT):
                bt = b * RT + rt
                lg, le = lds[rt]
                nc.vector.tensor_scalar_mul(out=lg[:], in0=lg[:],
                                            scalar1=attTf[:, rt, 0:1])
                nc.vector.tensor_scalar_mul(out=le[:], in0=le[:],
                                            scalar1=attTf[:, rt, 1:2])
                outs.append((bt, lg, le))
            return outs

        def emit_out_dmas(outs):
            for bt, og, oe in outs:
                nc.sync.dma_start(out=out[bt * 128 : (bt + 1) * 128, :], in_=og[:])
                nc.sync.dma_start(out=out[R + bt * 128 : R + (bt + 1) * 128, :],
                                  in_=oe[:])

        pending = None
        for b in range(NBLK):
            cur = emit_head(b)
            if b == 0:
                emit_w1_h1()
                emit_w1_h2()
            if pending is not None:
                emit_out_dmas(pending)
            pending = emit_tail(b, cur)
        emit_out_dmas(pending)


_CACHED = {}


def _get_graph():
    if "g" not in _CACHED:
        _CACHED["g"] = build_graph()
    return _CACHED["g"]


def kernel(**inputs):
    nc = _get_graph()
    in_maps = make_in_maps(inputs)
    res = run_bass_kernel_spmd(nc, in_maps, core_ids=list(range(N_CORES)))
    outs = [np.asarray(r["out"], np.float32) for r in res.results]
    out1 = np.concatenate([o[:R] for o in outs], axis=0)
    out2 = np.concatenate([o[R:] for o in outs], axis=0)
    return out1, out2


if __name__ == "__main__":
    nc = build_graph()
    print("graph built OK")
